# revision 13
# baseline (speedup 1.0000x reference)
"""Trainium2 Bass kernel for the DocRED-style segment_reduce model.

Sharding: 8 cores, data-parallel: core c -> (doc = c//2, pair-half = c%2).
Each core independently computes logits for its 256 pairs. No collectives.
All segment reductions / gathers are lowered to one-hot matmuls whose
one-hot matrices are built on the host from the integer inputs and passed
as per-core input tensors (the SPMD program itself is index-agnostic).

v2 restructure vs baseline:
  - fp16 everywhere (same engine speed as bf16, ~8x better mantissa)
  - P3 attention products read PE results straight from PSUM (fewer ACT
    copies); 4-head packs sized for DVE efficiency; part of the work
    offloaded to ACT / GpSimd
  - P4 (rel) runs as its own phase with the EW extractor matmuls issued
    in the P3->P4 dependency gap to keep the PE busy (p-state ramp)
  - P6 bilinear replication split between a PE one-hot route and a DMA
    broadcast route (tunable), b2 gathers via small DRAM reads
  - classifier weight stream + staging on separate DMA queues
"""

import os

import numpy as np

import concourse.bacc as bacc
import concourse.bass as bass
import concourse.mybir as mybir
import concourse.tile as tile
from concourse.bass_utils import run_bass_kernel_spmd

B, M, H = 4, 128, 1024
NH, L = 16, 1024
E, R = 64, 512
EMB, BS, NCL = 768, 64, 97
K12 = EMB // BS  # 12 blocks
NCORES = 8
RPC = R // 2  # pairs per core

F32 = mybir.dt.float32
F16 = mybir.dt.float16
BF16 = mybir.dt.bfloat16

# compute dtype mode: "f16" | "bf16"
MM_MODE = os.environ.get("DOCRED_MM_MODE", "f16")

# P3 per-pack product route (4 packs of 4 heads per lc). DVE/GpSimd can
# read at most ONE operand from PSUM, so:
#   "a": ACT copies the x-half to fp16 SBUF, DVE mults SBUF x PSUM (1cyc/el)
#   "b": ACT copies both halves,  DVE mults fp16 SBUF (0.5 cyc/el)
#   "g": ACT copies the x-half, GpSimd mults SBUF x PSUM
P3_ROUTES = os.environ.get("DOCRED_P3_ROUTES", "a,a,b,g").split(",")
# P6 k-block routes:
#   "pd": PE one-hot replication, DVE mults straight from PSUM (even k only)
#   "pa": PE replication + ACT copy to fp16, DVE mults fp16  (even k only)
#   "d":  DMA broadcast replication from DRAM staging, DVE mults
#   "g":  DMA broadcast replication, GpSimd mults (prefetched at P6 start)
P6_ROUTES = os.environ.get(
    "DOCRED_P6_ROUTES", "pd,d,pa,d,pd,d,pa,d,pa,d,d,g").split(",")
# P3: offload the first tree-add level of each lc to GpSimd
P3_GPS_ADD = os.environ.get("DOCRED_P3_GPS_ADD", "1") == "1"


def _fdt():
    return BF16 if MM_MODE == "bf16" else F16


def _np_fdt():
    import ml_dtypes

    return np.dtype(ml_dtypes.bfloat16) if MM_MODE == "bf16" else np.float16


class _Builder:
    def __init__(self, mm_mode: str):
        self.mm_mode = mm_mode
        self.fdt = {"f16": F16, "bf16": BF16}[mm_mode]
        nc = bacc.Bacc("TRN2", target_bir_lowering=False, debug=False)
        self.nc = nc
        fdt = self.fdt
        d = {}
        d["ent"] = nc.dram_tensor("ent", [M, H], F32, kind="ExternalInput")
        # lc-major attention layout: [m, lc, h, 128]
        d["attn"] = nc.dram_tensor("attn", [M, 8 * NH * 128], fdt,
                                   kind="ExternalInput")
        d["seq"] = nc.dram_tensor("seq", [128, 8 * (L + 1)], fdt,
                                  kind="ExternalInput")
        d["ssum"] = nc.dram_tensor("ssum", [M, E], fdt, kind="ExternalInput")
        d["ohxy2"] = nc.dram_tensor("ohxy2", [M, 2 * RPC], fdt,
                                    kind="ExternalInput")
        d["eadd"] = nc.dram_tensor("eadd", [E, 1], F32, kind="ExternalInput")
        d["ohx"] = nc.dram_tensor("ohx", [E, RPC], fdt, kind="ExternalInput")
        d["ohy"] = nc.dram_tensor("ohy", [E, RPC], fdt, kind="ExternalInput")
        d["wh"] = nc.dram_tensor("wh", [128, 16 * EMB], fdt, kind="ExternalInput")
        d["wt"] = nc.dram_tensor("wt", [128, 16 * EMB], fdt, kind="ExternalInput")
        d["bh"] = nc.dram_tensor("bh", [128, EMB // 128], F32, kind="ExternalInput")
        d["bt"] = nc.dram_tensor("bt", [128, EMB // 128], F32, kind="ExternalInput")
        d["wb"] = nc.dram_tensor("wb", [128, 384 * NCL], fdt, kind="ExternalInput")
        d["bbc"] = nc.dram_tensor("bbc", [NCL, 1], F32, kind="ExternalInput")
        d["ident"] = nc.dram_tensor("ident", [128, 128], fdt, kind="ExternalInput")
        d["repm"] = nc.dram_tensor("repm", [E, 32 * 128], fdt, kind="ExternalInput")
        d["lt"] = nc.dram_tensor("lt", [NCL, RPC], F32, kind="ExternalOutput")
        self.d = d
        with tile.TileContext(nc) as tc:
            self.build(tc)
        nc.compile()

    def mm(self, out, lhsT, rhs, **kw):
        return self.nc.tensor.matmul(out, lhsT, rhs, **kw)

    def tp(self, out, in_, ident, **kw):
        return self.nc.tensor.matmul(out, in_, ident, is_transpose=True, **kw)

    def build(self, tc):
        nc = self.nc
        d = self.d
        fdt = self.fdt
        AF = mybir.ActivationFunctionType

        with (
            tc.tile_pool(name="pin", bufs=1) as pin,
            tc.tile_pool(name="mid", bufs=1) as mid,
            tc.tile_pool(name="prodp", bufs=2) as prodp,
            tc.tile_pool(name="dramp", bufs=1, space="DRAM") as dramp,
        ):
            # ---------- persistent small tensors ----------
            ident = pin.tile([128, 128], fdt)
            ssum = pin.tile([M, E], fdt)
            ohxy2 = pin.tile([M, 2 * RPC], fdt)
            eadd = pin.tile([E, 1], F32)
            ohx = pin.tile([E, RPC], fdt)
            ohy = pin.tile([E, RPC], fdt)
            bh = pin.tile([128, EMB // 128], F32)
            bt = pin.tile([128, EMB // 128], F32)
            bbc = pin.tile([NCL, 1], F32)
            repm = pin.tile([E, 32, 128], fdt)
            for t, key in [
                (ident, "ident"), (ssum, "ssum"), (ohxy2, "ohxy2"),
                (eadd, "eadd"), (ohx, "ohx"), (ohy, "ohy"),
                (bh, "bh"), (bt, "bt"), (bbc, "bbc"),
            ]:
                nc.sync.dma_start(t[:], d[key].ap())
            nc.sync.dma_start(repm[:], d["repm"].ap()
                              .rearrange("p (a b) -> p a b", a=32))

            ent = mid.tile([M, H], F32)
            nc.sync.dma_start(ent[:], d["ent"].ap())

            # attn: stream lc-major so P3 can start on slice 0
            attn_cm = tc.tile_pool(name="attnp", bufs=1)
            attnp = attn_cm.__enter__()
            attn = attnp.tile([M, 8, NH, 128], fdt)
            av = d["attn"].ap().rearrange("p (lc h f) -> p lc h f", lc=8, h=NH)
            for lc in range(8):
                nc.sync.dma_start(attn[:, lc], av[:, lc])

            seq_cm = tc.tile_pool(name="seqp", bufs=1)
            seqp = seq_cm.__enter__()
            sq = seqp.tile([128, 8, L + 1], fdt)
            wpin_cm = tc.tile_pool(name="wpin", bufs=1)
            wpin = wpin_cm.__enter__()
            wh_sb = wpin.tile([128, 16, EMB], fdt, name="wh_sb")
            wt_sb = wpin.tile([128, 16, EMB], fdt, name="wt_sb")
            nc.sync.dma_start(
                wh_sb[:], d["wh"].ap().rearrange("p (a b) -> p a b", a=16))
            nc.sync.dma_start(
                wt_sb[:], d["wt"].ap().rearrange("p (a b) -> p a b", a=16))
            nc.sync.dma_start(sq[:], d["seq"].ap()
                              .rearrange("p (a b) -> p a b", a=8))

            # ---------- P1: exp + segment-sum + log ----------
            psA_cm = tc.tile_pool(name="psA", bufs=1, space="PSUM")
            psA = psA_cm.__enter__()
            pexp = mid.tile([M, H], fdt, name="pexp")
            nc.scalar.activation(pexp[:], ent[:], AF.Exp)
            ps_ent = psA.tile([E, H], F32)
            for nh in range(2):
                self.mm(ps_ent[:, nh * 512:(nh + 1) * 512], ssum[:],
                        pexp[:, nh * 512:(nh + 1) * 512])
            ent_sb = mid.tile([E, H], fdt)
            nc.scalar.activation(ent_sb[:], ps_ent[:], AF.Ln, bias=eadd[:])
            psA_cm.__exit__(None, None, None)

            # entT: [h-part, hc, e]
            psT_cm = tc.tile_pool(name="psT", bufs=2, space="PSUM")
            psT = psT_cm.__enter__()
            entT = mid.tile([128, 8, E], fdt, name="entT")
            for hc in range(8):
                ps_t2 = psT.tile([128, E], fdt, tag="tp")
                self.tp(ps_t2[:], ent_sb[:, hc * 128:(hc + 1) * 128],
                        ident[0:E, 0:E])
                nc.scalar.copy(entT[:, hc, :], ps_t2[:])
            psT_cm.__exit__(None, None, None)

            # ---------- P3: C = sum_h gather_x(attn_h) * gather_y(attn_h) --
            CTmm = mid.tile([128, 8, RPC], fdt, name="CTmm")
            psP_cm = tc.tile_pool(name="psP", bufs=1, space="PSUM")
            psP = psP_cm.__enter__()
            for lc in range(8):
                prods = []
                for q in range(4):
                    psq = psP.tile([128, 4, 2, RPC], F32, tag="p3", bufs=2,
                                   name="psq")
                    for hh in range(4):
                        a_sl = attn[:, lc, q * 4 + hh, :]
                        self.mm(psq[:, hh], a_sl, ohxy2[:])
                    prod = prodp.tile([128, 4, RPC], fdt, tag=f"prod{q}",
                                      bufs=2, name=f"prod{q}")
                    route = P3_ROUTES[q]
                    if route in ("b", "g"):
                        # GPSIMD cannot read PSUM at all; "b"/"g" copy both
                        # halves to fp16 SBUF first
                        g16 = prodp.tile([128, 4, 2, RPC], fdt, tag="g16",
                                         bufs=2, name="g16")
                        nc.scalar.copy(g16[:], psq[:])
                        eng = nc.gpsimd if route == "g" else nc.vector
                        eng.tensor_mul(prod[:], g16[:, :, 0, :],
                                       g16[:, :, 1, :])
                    else:
                        gx = prodp.tile([128, 4, RPC], fdt, tag="gx",
                                        bufs=2, name="gx")
                        nc.scalar.copy(gx[:], psq[:, :, 0, :])
                        nc.vector.tensor_mul(prod[:], gx[:], psq[:, :, 1, :])
                    prods.append(prod)
                # tree reduce 4 tiles -> CTmm[:, lc, :]
                if P3_GPS_ADD:
                    nc.gpsimd.tensor_add(prods[0][:], prods[0][:], prods[1][:])
                else:
                    nc.vector.tensor_add(prods[0][:], prods[0][:], prods[1][:])
                nc.vector.tensor_add(prods[2][:], prods[2][:], prods[3][:])
                nc.vector.tensor_add(prods[0][:], prods[0][:], prods[2][:])
                f2 = prodp.tile([128, 2, RPC], fdt, tag="fold", bufs=2,
                                name="fold")
                nc.vector.tensor_add(f2[:], prods[0][:, 0:2, :],
                                     prods[0][:, 2:4, :])
                nc.vector.tensor_add(CTmm[:, lc, :], f2[:, 0, :], f2[:, 1, :])
            psP_cm.__exit__(None, None, None)

            # ---------- EW = ent_sb @ W[0:1024] (fills the P3->P4 gap) ----
            psEW_cm = tc.tile_pool(name="psEW", bufs=2, space="PSUM")
            psEW = psEW_cm.__enter__()
            EWh = mid.tile([E, EMB], fdt, name="EWh")
            EWt = mid.tile([E, EMB], fdt, name="EWt")
            for w, ew in ((wh_sb, EWh), (wt_sb, EWt)):
                ps_ew = psEW.tile([E, EMB], F32, tag="ew")
                for hc in range(8):
                    for lo, hi in ((0, 512), (512, 768)):
                        self.mm(ps_ew[:, lo:hi], entT[:, hc, :],
                                w[:, hc, lo:hi],
                                start=(hc == 0), stop=(hc == 7))
                nc.scalar.copy(ew[:], ps_ew[:])
            psEW_cm.__exit__(None, None, None)

            # ---------- P4: rel = normalize(C) @ seq ----------
            psR_cm = tc.tile_pool(name="psR", bufs=1, space="PSUM")
            psR = psR_cm.__enter__()
            ps_rel = [psR.tile([128, L], F32, name=f"ps_rel{i}")
                      for i in range(2)]
            ps_s8 = psR.tile([128, 2, 8], F32, name="ps_s8")
            for lc in range(8):
                st, sp = lc == 0, lc == 7
                for rc in range(2):
                    lhsT = CTmm[:, lc, rc * 128:(rc + 1) * 128]
                    self.mm(ps_rel[rc][:, 0:512], lhsT, sq[:, lc, 0:512],
                            start=st, stop=sp)
                    self.mm(ps_rel[rc][:, 512:1024], lhsT, sq[:, lc, 512:1024],
                            start=st, stop=sp)
                    self.mm(ps_s8[:, rc, lc:lc + 1], lhsT,
                            sq[:, lc, 1024:1025], start=True, stop=True)
            relT = mid.tile([128, 8, RPC], fdt, name="relT")
            psT2_cm = tc.tile_pool(name="psT2", bufs=2, space="PSUM")
            psT2 = psT2_cm.__enter__()
            for rc in range(2):
                tdenom = prodp.tile([128, 1], F32, tag="tden")
                nc.vector.tensor_reduce(tdenom[:], ps_s8[:, rc, :],
                                        axis=mybir.AxisListType.X,
                                        op=mybir.AluOpType.add)
                nc.scalar.activation(tdenom[:], tdenom[:], AF.Copy,
                                     bias=16e-5, scale=1.0)
                frec = prodp.tile([128, 1], F32, tag="frec")
                nc.vector.reciprocal(frec[:], tdenom[:])
                rel_sc = mid.tile([128, L], fdt, tag="rel_sc", name="rel_sc")
                nc.vector.tensor_scalar_mul(rel_sc[:], ps_rel[rc][:], frec[:])
                for dc in range(8):
                    ps_t = psT2.tile([128, 128], fdt, tag="tp2")
                    self.tp(ps_t[:], rel_sc[:, dc * 128:(dc + 1) * 128],
                            ident[:])
                    nc.scalar.copy(relT[:, dc, rc * 128:(rc + 1) * 128],
                                   ps_t[:])
            psT2_cm.__exit__(None, None, None)
            psR_cm.__exit__(None, None, None)

            # ---------- P5: extractors -> hsEt/tsEt [emb, n] ----------
            psE_cm = tc.tile_pool(name="psE", bufs=4, space="PSUM")
            psE = psE_cm.__enter__()
            hsEt = mid.tile([128, 6, RPC], fdt, name="hsEt")
            tsEt = mid.tile([128, 6, RPC], fdt, name="tsEt")
            hsd = dramp.tile([128, 6, RPC], fdt, name="hsd")
            tsd = dramp.tile([128, 6, RPC], fdt, name="tsd")
            for ec in range(6):
                for (w, bvec, ew, oh, dst, dstd) in (
                    (wh_sb, bh, EWh, ohx, hsEt, hsd),
                    (wt_sb, bt, EWt, ohy, tsEt, tsd),
                ):
                    ps_e = psE.tile([128, RPC], F32, tag="pe", name="ps_e")
                    self.mm(ps_e[:], ew[:, ec * 128:(ec + 1) * 128], oh[:],
                            start=True, stop=False)
                    for kc in range(8, 16):
                        self.mm(ps_e[:], w[:, kc, ec * 128:(ec + 1) * 128],
                                relT[:, kc % 8, :],
                                start=False, stop=(kc == 15))
                    nc.scalar.activation(dst[:, ec, :], ps_e[:], AF.Tanh,
                                         bias=bvec[:, ec:ec + 1])
                    nc.scalar.dma_start(dstd[:, ec, :], dst[:, ec, :])
            psE_cm.__exit__(None, None, None)
            wpin_cm.__exit__(None, None, None)
            seq_cm.__exit__(None, None, None)
            attn_cm.__exit__(None, None, None)

            # ---------- P6: block bilinear + classifier ----------
            with (
                tc.tile_pool(name="blph", bufs=1) as blph,
                tc.tile_pool(name="ps_lt", bufs=1, space="PSUM") as ps_lt,
                tc.tile_pool(name="psRep", bufs=2, space="PSUM") as psRep,
            ):
                pslt = ps_lt.tile([NCL, RPC], F32)

                def issue_b2t(k, tag="b2t", bufs=3):
                    kk = 64 * (k % 2)
                    ec = k // 2
                    b2t = blph.tile([128, RPC], fdt, tag=tag, bufs=bufs,
                                    name=tag)
                    for h0 in (0, 1):
                        nc.sync.dma_start(b2t[64 * h0:64 * (h0 + 1)],
                                          tsd[kk:kk + 64, ec, :])
                    return b2t

                def issue_b1rep(k, bufs=2, tag="b1rep"):
                    kk = 64 * (k % 2)
                    ec = k // 2
                    b1rep = blph.tile([128, 32, RPC], fdt, tag=tag, bufs=bufs,
                                      name=tag)
                    # dest [64, 32, n] per i-half; src rows broadcast across
                    # the 64 j-partitions
                    for h0 in (0, 1):
                        src = hsd[kk + 32 * h0:kk + 32 * (h0 + 1), ec, :] \
                            .unsqueeze(0).broadcast_to([64, 32, RPC])
                        nc.sync.dma_start(b1rep[64 * h0:64 * (h0 + 1)], src)
                    return b1rep

                # prefetch GPS-route ks fully (they multiply slowly on
                # gpsimd while the rest of P6 proceeds)
                gps_blT = {}
                for k in range(K12):
                    if P6_ROUTES[k] != "g":
                        continue
                    b2t = issue_b2t(k, tag=f"b2g{k}", bufs=1)
                    b1rep = issue_b1rep(k, bufs=1, tag=f"b1g{k}")
                    blT = blph.tile([128, 32, RPC], fdt, tag=f"blg{k}",
                                    bufs=1, name=f"blg{k}")
                    b2b = b2t[:].unsqueeze(1).broadcast_to([128, 8, RPC])
                    for g in range(4):
                        nc.gpsimd.tensor_mul(blT[:, g * 8:(g + 1) * 8, :],
                                             b1rep[:, g * 8:(g + 1) * 8, :],
                                             b2b)
                    gps_blT[k] = blT

                cg = 0
                for k in range(K12):
                    kk = 64 * (k % 2)
                    ec = k // 2
                    route = P6_ROUTES[k]
                    wb = blph.tile([128, 32 * NCL], fdt, tag="wb", bufs=3,
                                   name="wb")
                    nc.sync.dma_start(
                        wb[:], d["wb"].ap()[:, k * 32 * NCL:(k + 1) * 32 * NCL])
                    if route == "g":
                        blT = gps_blT[k]
                    elif route in ("pd", "pa"):
                        assert kk == 0, "PE replication route needs even k"
                        b2t = issue_b2t(k)
                        blT = blph.tile([128, 32, RPC], fdt, tag="blT",
                                        bufs=2, name="blT")
                        hsE64 = hsEt[kk:kk + 64, ec, :]
                        b2b = b2t[:].unsqueeze(1).broadcast_to([128, 4, RPC])
                        for cq in range(8):
                            psq6 = psRep.tile([128, 4, RPC], F32, tag="rep",
                                              bufs=2, name="psq6")
                            for i4 in range(4):
                                self.mm(psq6[:, i4, :],
                                        repm[:, cq * 4 + i4, :], hsE64)
                            if route == "pa":
                                b1c = blph.tile([128, 4, RPC], fdt, tag="b1c",
                                                bufs=3, name="b1c")
                                nc.scalar.copy(b1c[:], psq6[:])
                                nc.vector.tensor_mul(
                                    blT[:, cq * 4:(cq + 1) * 4, :],
                                    b1c[:], b2b)
                            else:
                                nc.vector.tensor_mul(
                                    blT[:, cq * 4:(cq + 1) * 4, :],
                                    psq6[:], b2b)
                    else:
                        b2t = issue_b2t(k)
                        b1rep = issue_b1rep(k)
                        blT = blph.tile([128, 32, RPC], fdt, tag="blT",
                                        bufs=2, name="blT")
                        b2b = b2t[:].unsqueeze(1).broadcast_to([128, 8, RPC])
                        for g in range(4):
                            nc.vector.tensor_mul(
                                blT[:, g * 8:(g + 1) * 8, :],
                                b1rep[:, g * 8:(g + 1) * 8, :], b2b)
                    for c in range(32):
                        self.mm(pslt[:], wb[:, c * NCL:(c + 1) * NCL],
                                blT[:, c, :],
                                start=(cg == 0), stop=(cg == 383))
                        cg += 1

                out_sb = mid.tile([NCL, RPC], F32)
                nc.scalar.activation(out_sb[:], pslt[:], AF.Identity,
                                     bias=bbc[:])
                nc.sync.dma_start(d["lt"].ap(), out_sb[:])


_PROGRAM_CACHE = {}


def _get_program(mm_mode: str):
    if mm_mode not in _PROGRAM_CACHE:
        _PROGRAM_CACHE[mm_mode] = _Builder(mm_mode)
    return _PROGRAM_CACHE[mm_mode]


def _host_inputs(seq_lhs, ent_lhs, ent_to_seq_attn, entity_id_labels, hts,
                 Wh, bh, Wt, bt, Wb, bb):
    """Build the 8 per-core input maps (all host-side numpy)."""
    fdt = _np_fdt()
    seq_lhs = np.asarray(seq_lhs, np.float32)
    ent_lhs = np.asarray(ent_lhs, np.float32)
    ent_to_seq_attn = np.asarray(ent_to_seq_attn, np.float32)
    entity_id_labels = np.asarray(entity_id_labels)
    hts = np.asarray(hts)
    Wh = np.asarray(Wh, np.float32)
    Wt = np.asarray(Wt, np.float32)
    Wb = np.asarray(Wb, np.float32)
    bh = np.asarray(bh, np.float32)
    bt = np.asarray(bt, np.float32)
    bb = np.asarray(bb, np.float32)

    # device chunk (k, c) row p maps to Wb row k*4096 + i*64 + j with
    # i = c + 32*(p//64), j = p%64
    p_ = np.arange(128)
    c_ = np.arange(32)
    k_ = np.arange(K12)
    rows = (k_[:, None, None] * 4096
            + (c_[None, :, None] + 32 * (p_[None, None, :] // 64)) * 64
            + (p_[None, None, :] % 64))  # [k, c, p]
    wb_r = np.ascontiguousarray(
        Wb[rows.reshape(-1), :].reshape(K12 * 32, 128, NCL)
        .transpose(1, 0, 2).reshape(128, 384 * NCL)
    ).astype(fdt)
    wh_c = np.ascontiguousarray(
        Wh.reshape(16, 128, EMB).transpose(1, 0, 2).reshape(128, 16 * EMB)
    ).astype(fdt)
    wt_c = np.ascontiguousarray(
        Wt.reshape(16, 128, EMB).transpose(1, 0, 2).reshape(128, 16 * EMB)
    ).astype(fdt)
    bh_c = np.ascontiguousarray(bh.reshape(EMB // 128, 128).T)
    bt_c = np.ascontiguousarray(bt.reshape(EMB // 128, 128).T)
    bb_c = np.ascontiguousarray(bb.reshape(NCL, 1))
    ident = np.eye(128, dtype=np.float32).astype(fdt)
    # repm[r, c, p] = 1 iff r == c + 32*(p//64)
    repm_h = np.zeros((E, 32, 128), np.float32)
    for c in range(32):
        repm_h[c, c, 0:64] = 1.0
        repm_h[c + 32, c, 64:128] = 1.0
    repm_h = repm_h.reshape(E, 32 * 128).astype(fdt)

    in_maps = []
    for c in range(NCORES):
        doc, half = divmod(c, 2)
        sl = slice(half * RPC, (half + 1) * RPC)
        labels = entity_id_labels[doc].astype(np.int64)
        cnt = np.bincount(labels, minlength=E).astype(np.float32)
        S = np.zeros((M, E), np.float32)
        S[np.arange(M), labels] = 1.0
        smean = S / np.maximum(cnt, 1.0)[None, :]  # [M, E]
        eadd = (cnt == 0).astype(np.float32).reshape(E, 1)
        hi = hts[doc, sl, 0].astype(np.int64)
        ti = hts[doc, sl, 1].astype(np.int64)
        ohx = np.zeros((E, RPC), np.float32)
        ohx[hi, np.arange(RPC)] = 1.0
        ohy = np.zeros((E, RPC), np.float32)
        ohy[ti, np.arange(RPC)] = 1.0
        ohxy2 = np.concatenate([smean @ ohx, smean @ ohy], axis=1)  # [M, 512]
        attn = np.ascontiguousarray(
            ent_to_seq_attn[doc].transpose(1, 0, 2)  # [M, NH, L]
            .reshape(M, NH, 8, 128).transpose(0, 2, 1, 3)  # [M, 8, NH, 128]
            .reshape(M, 8 * NH * 128)
        ).astype(fdt)
        seq_r = seq_lhs[doc].reshape(8, 128, L).transpose(1, 0, 2)
        seq_aug = np.concatenate(
            [seq_r, np.ones((128, 8, 1), np.float32)], axis=2
        )
        in_maps.append({
            "ent": np.ascontiguousarray(ent_lhs[doc]),
            "attn": attn,
            "seq": np.ascontiguousarray(
                seq_aug.reshape(128, 8 * (L + 1))).astype(fdt),
            "ssum": S.astype(fdt),
            "ohxy2": ohxy2.astype(fdt),
            "eadd": eadd,
            "ohx": ohx.astype(fdt),
            "ohy": ohy.astype(fdt),
            "wh": wh_c, "wt": wt_c, "bh": bh_c, "bt": bt_c,
            "wb": wb_r, "bbc": bb_c, "ident": ident, "repm": repm_h,
        })
    return in_maps


_LAST_RESULTS = {}


def kernel(**inputs) -> np.ndarray:
    prog = _get_program(MM_MODE)
    in_maps = _host_inputs(**inputs)
    trace = os.environ.get("DOCRED_TRACE", "0") == "1"
    res = run_bass_kernel_spmd(
        prog.nc, in_maps, core_ids=list(range(NCORES)), trace=trace,
    )
    _LAST_RESULTS["res"] = res
    out = np.empty((B * R, NCL), np.float32)
    for c in range(NCORES):
        doc, half = divmod(c, 2)
        lt = res.results[c]["lt"]  # [NCL, RPC]
        out[doc * R + half * RPC: doc * R + (half + 1) * RPC, :] = lt.T
    return out
